# revision 9
# baseline (speedup 1.0000x reference)
"""MLA (multi-head latent attention) Trainium2 kernel, 8-core tensor-parallel
over heads. Self-contained: hardcodes shapes for
B=1, S=2048, D=4096, NH=32, NKV=8, HD=128, KV_RANK=512.

Sharding: core c owns query heads [4c, 4c+4) and KV head c (GQA N_REP=4 means
those 4 query heads share exactly one KV head). W_DKV@W_UK / W_DKV@W_UV are
fused on-device so K^T/V^T come straight from x^T; each core also emits a
64-column slice of c_KV. The Wo partial products are summed on the host.
"""
import math
import sys

sys.path.insert(0, "/opt/trn_rl_repo")

import numpy as np

import concourse.bacc as bacc
import concourse.mybir as mybir
import concourse.tile as tile
from concourse import bass_utils

P = 128
S = 2048
D = 4096
KV = 512
NH = 32
NKV = 8
HD = 128
NHC = 4              # query heads per core
NCORES = 8
DT = D // P          # 32 d-tiles
CT = KV // P         # 4 rank-tiles
SC = S // 512        # 4 s-chunks
KT = S // P          # 16 k-tiles
ST = S // P          # 16 s-tiles
DC = D // 512        # 8 output column chunks
SCALE = 1.0 / math.sqrt(HD)
BIG_NEG = -60000.0

F32R = mybir.dt.float32r
F32 = mybir.dt.float32
AF = mybir.ActivationFunctionType

_CACHE = {}


def _build_nc():
    nc = bacc.Bacc("TRN2", target_bir_lowering=False, debug=False,
                   num_devices=NCORES)

    d_xt = nc.dram_tensor("xt", (D, S), F32R, kind="ExternalInput").ap()
    d_wq = nc.dram_tensor("wq", (D, 512), F32R, kind="ExternalInput").ap()
    d_wdkvT = nc.dram_tensor("wdkvT", (KV, D), F32R, kind="ExternalInput").ap()
    d_wukv = nc.dram_tensor("wukv", (KV, 256), F32R, kind="ExternalInput").ap()
    # pre-tiled image: [p, dt*64 + c] = W_DKV[dt*128 + p, core*64 + c]
    d_wdkvsl = nc.dram_tensor("wdkvsl", (P, DT * 64), F32R,
                              kind="ExternalInput").ap()
    d_wo = nc.dram_tensor("wo", (512, D), F32R, kind="ExternalInput").ap()
    d_cmap = nc.dram_tensor("cmap", (P, S), F32R, kind="ExternalInput").ap()
    d_smap = nc.dram_tensor("smap", (P, S), F32R, kind="ExternalInput").ap()
    d_masks = nc.dram_tensor("masks", (P, 4 * 512), F32R,
                             kind="ExternalInput").ap()
    # consts image: cols [0,128) identity, [128,256) rope-swap perm,
    # col 256 ones-column; row 0 cols [257,385) ones-row
    d_consts = nc.dram_tensor("consts", (P, 385), F32R,
                              kind="ExternalInput").ap()

    d_out = nc.dram_tensor("outp", (S, D), F32, kind="ExternalOutput").ap()
    d_ckvT = nc.dram_tensor("ckvT", (64, S), F32, kind="ExternalOutput").ap()

    with tile.TileContext(nc) as tc:
        with tc.tile_pool(name="g_const", bufs=1) as gconst, \
             tc.tile_pool(name="persist", bufs=1) as persist:
            consts = gconst.tile([P, 385], F32R)
            nc.sync.dma_start(out=consts, in_=d_consts)
            ident = consts[:, 0:128]
            rswap = consts[:, 128:256]
            onec = consts[:, 256:257]
            oner = consts[0:1, 257:385]

            qT = persist.tile([P, NHC, S], F32R)     # roped Q^T per head
            kT = persist.tile([P, S], F32R)          # roped K^T
            vT = persist.tile([P, S], F32R)          # V^T (pre-transpose)

            # ---------------- Phase 0 + A weights ----------------
            pAw_ctx = tc.tile_pool(name="pA_w", bufs=1)
            with pAw_ctx as pAw:
                wq_sb = pAw.tile([P, DT, 512], F32R)
                for dt in range(DT):
                    nc.sync.dma_start(out=wq_sb[:, dt, :],
                                      in_=d_wq[dt * P:(dt + 1) * P, :])
                wkvf_sb = pAw.tile([P, DT, 256], F32R)
                wdkvsl_sb = pAw.tile([P, DT, 64], F32R)
                nc.sync.dma_start(
                    out=wdkvsl_sb,
                    in_=d_wdkvsl.rearrange("p (a c) -> p a c", a=DT))

                # Phase 0: WKV_fused[d, :] = sum_c W_DKV[d, c] * [W_UK|W_UV][c, :]
                with tc.tile_pool(name="ph0", bufs=3) as ph0, \
                     tc.tile_pool(name="wukvp", bufs=1) as wukvp, \
                     tc.tile_pool(name="ps0", bufs=2, space="PSUM") as ps0:
                    wukv_sb = wukvp.tile([P, CT, 256], F32R)
                    nc.sync.dma_start(
                        out=wukv_sb,
                        in_=d_wukv.rearrange("(a p) m -> p a m", p=P))
                    wdkvT_v = d_wdkvT.rearrange("(a p) d -> p a d", p=P)
                    for dtb in range(8):
                        wt = ph0.tile([P, CT, 512], F32R, tag="wdkvT")
                        nc.sync.dma_start(
                            out=wt, in_=wdkvT_v[:, :, dtb * 512:(dtb + 1) * 512])
                        for j in range(4):
                            dt = dtb * 4 + j
                            ps = ps0.tile([P, 256], F32, tag="ps0")
                            for ct in range(CT):
                                nc.tensor.matmul(
                                    ps,
                                    lhsT=wt[:, ct, j * P:(j + 1) * P],
                                    rhs=wukv_sb[:, ct, :],
                                    start=(ct == 0), stop=(ct == CT - 1))
                            nc.scalar.copy(wkvf_sb[:, dt, :], ps)

                # Phase A: stream x^T, project Q/K/V + c_KV slice, apply rope
                with tc.tile_pool(name="xtp", bufs=3) as xtp, \
                     tc.tile_pool(name="mapp", bufs=2) as mapp, \
                     tc.tile_pool(name="ropep", bufs=1) as ropep, \
                     tc.tile_pool(name="stg", bufs=2) as stg, \
                     tc.tile_pool(name="psA", bufs=1, space="PSUM") as psA, \
                     tc.tile_pool(name="psR", bufs=1, space="PSUM") as psR:
                    for sc in range(SC):
                        s0 = sc * 512
                        qps = [psA.tile([P, 512], F32, tag=f"qps{h}",
                                        name=f"qps{h}_{sc}")
                               for h in range(NHC)]
                        kps = psA.tile([P, 512], F32, tag="kps")
                        vps = psA.tile([P, 512], F32, tag="vps")
                        cps = psA.tile([64, 512], F32, tag="cps")
                        for dt in range(DT):
                            xt_t = xtp.tile([P, 512], F32R, tag="xt")
                            nc.sync.dma_start(
                                out=xt_t,
                                in_=d_xt[dt * P:(dt + 1) * P, s0:s0 + 512])
                            st_f = (dt == 0)
                            sp_f = (dt == DT - 1)
                            for h in range(NHC):
                                nc.tensor.matmul(
                                    qps[h],
                                    lhsT=wq_sb[:, dt, h * P:(h + 1) * P],
                                    rhs=xt_t, start=st_f, stop=sp_f)
                            nc.tensor.matmul(kps, lhsT=wkvf_sb[:, dt, 0:128],
                                             rhs=xt_t, start=st_f, stop=sp_f)
                            nc.tensor.matmul(vps, lhsT=wkvf_sb[:, dt, 128:256],
                                             rhs=xt_t, start=st_f, stop=sp_f)
                            nc.tensor.matmul(cps, lhsT=wdkvsl_sb[:, dt, :],
                                             rhs=xt_t, start=st_f, stop=sp_f)
                        # c_KV slice out (fp32)
                        c_sb = stg.tile([64, 512], F32, tag="ckv")
                        nc.scalar.copy(c_sb, cps)
                        nc.sync.dma_start(out=d_ckvT[:, s0:s0 + 512], in_=c_sb)
                        # V^T evict (no rope)
                        nc.scalar.copy(vT[:, s0:s0 + 512], vps)
                        # rope: dest = cmap*raw + smap*(rswap @ raw)
                        cm = mapp.tile([P, 512], F32R, tag="cm")
                        nc.sync.dma_start(out=cm, in_=d_cmap[:, s0:s0 + 512])
                        sm = mapp.tile([P, 512], F32R, tag="sm")
                        nc.sync.dma_start(out=sm, in_=d_smap[:, s0:s0 + 512])
                        streams = [(qps[h], qT[:, h, s0:s0 + 512])
                                   for h in range(NHC)]
                        streams.append((kps, kT[:, s0:s0 + 512]))
                        for psrc, dest in streams:
                            raw = ropep.tile([P, 512], F32R, tag="raw",
                                             bufs=2)
                            nc.scalar.copy(raw, psrc)
                            rps = psR.tile([P, 512], F32, tag="rot")
                            nc.tensor.matmul(rps, lhsT=rswap, rhs=raw,
                                             start=True, stop=True)
                            rot = ropep.tile([P, 512], F32R, tag="rot_sb")
                            nc.scalar.copy(rot, rps)
                            t1 = ropep.tile([P, 512], F32R, tag="t1")
                            nc.vector.tensor_mul(t1, raw, cm)
                            t2 = ropep.tile([P, 512], F32R, tag="t2")
                            nc.vector.tensor_mul(t2, rot, sm)
                            nc.vector.tensor_add(dest, t1, t2)

            # ---------------- Phase B: attention ----------------
            pBp_cm = tc.tile_pool(name="pB_persist", bufs=1)
            pBp = pBp_cm.__enter__()
            attnT = pBp.tile([P, NHC, S], F32R)
            wo_sb = pBp.tile([P, 4, D], F32R)
            for ht in range(4):
                nc.sync.dma_start(out=wo_sb[:, ht, :],
                                  in_=d_wo[ht * P:(ht + 1) * P, :])
            masks_sb = pBp.tile([P, 4, 512], F32R)
            nc.sync.dma_start(out=masks_sb,
                              in_=d_masks.rearrange("p (a m) -> p a m", a=4))
            v_sb = pBp.tile([P, KT, P], F32R)

            with tc.tile_pool(name="ptp", bufs=3) as ptp, \
                 tc.tile_pool(name="obp", bufs=2) as obp, \
                 tc.tile_pool(name="lrp", bufs=2) as lrp, \
                 tc.tile_pool(name="psB", bufs=1, space="PSUM") as psB:
                # prologue: transpose V^T tiles -> V natural
                for kt in range(KT):
                    tp = psB.tile([P, P], F32R, tag="tr", bufs=1)
                    nc.tensor.transpose(tp, vT[:, kt * P:(kt + 1) * P], ident)
                    nc.scalar.copy(v_sb[:, kt, :], tp)

                for h in range(NHC):
                    for qc in range(SC):
                        q0 = qc * 512
                        kts = 4 * (qc + 1)
                        oacc = psB.tile([P, 512], F32, tag="o", bufs=2)
                        lacc = psB.tile([1, 512], F32, tag="l", bufs=1)
                        for kt in range(kts):
                            sps = psB.tile([P, 512], F32, tag="s", bufs=3)
                            diag = (kt >= 4 * qc)
                            nc.tensor.matmul(
                                sps, lhsT=kT[:, kt * P:(kt + 1) * P],
                                rhs=qT[:, h, q0:q0 + 512],
                                start=True, stop=(not diag))
                            if diag:
                                nc.tensor.matmul(
                                    sps, lhsT=ident,
                                    rhs=masks_sb[:, kt - 4 * qc, :],
                                    start=False, stop=True)
                            pt = ptp.tile([P, 512], F32R, tag="pt")
                            nc.scalar.activation(pt, sps, AF.Exp, scale=SCALE)
                            nc.tensor.matmul(oacc, lhsT=v_sb[:, kt, :], rhs=pt,
                                             start=(kt == 0),
                                             stop=(kt == kts - 1))
                            nc.tensor.matmul(lacc, lhsT=onec, rhs=pt,
                                             start=(kt == 0),
                                             stop=(kt == kts - 1))
                        # r = 1/l via exp(-ln(l)); broadcast via K=1 matmul
                        lnl = lrp.tile([1, 512], F32, tag="lnl")
                        nc.scalar.activation(lnl, lacc, AF.Ln)
                        rr = lrp.tile([1, 512], F32R, tag="rr")
                        nc.scalar.activation(rr, lnl, AF.Exp, scale=-1.0)
                        rbc = psB.tile([P, 512], F32, tag="rbc", bufs=1)
                        nc.tensor.matmul(rbc, lhsT=oner, rhs=rr,
                                         start=True, stop=True)
                        o_sb = obp.tile([P, 512], F32, tag="osb")
                        nc.scalar.copy(o_sb, oacc)
                        nc.vector.tensor_mul(attnT[:, h, q0:q0 + 512],
                                             o_sb, rbc)

            # ---------------- Phase C: out = attn @ Wo ----------------
            with tc.tile_pool(name="ocp", bufs=4) as ocp, \
                 tc.tile_pool(name="psC", bufs=4, space="PSUM") as psC:
                for st in range(ST):
                    for dc in range(DC):
                        ps = psC.tile([P, 512], F32, tag="c")
                        for ht in range(4):
                            nc.tensor.matmul(
                                ps,
                                lhsT=attnT[:, ht, st * P:(st + 1) * P],
                                rhs=wo_sb[:, ht, dc * 512:(dc + 1) * 512],
                                start=(ht == 0), stop=(ht == 3))
                        o2 = ocp.tile([P, 512], F32, tag="oc")
                        nc.any.tensor_copy(o2, ps)
                        nc.sync.dma_start(
                            out=d_out[st * P:(st + 1) * P,
                                      dc * 512:(dc + 1) * 512],
                            in_=o2)
            pBp_cm.__exit__(None, None, None)

    nc.compile()
    return nc


def _prep_in_maps(x, freqs_cis, W_DKV, W_UK, W_UV, Wq, Wo):
    x = np.asarray(x, np.float32)
    freqs_cis = np.asarray(freqs_cis, np.float32)
    W_DKV = np.asarray(W_DKV, np.float32)
    W_UK = np.asarray(W_UK, np.float32)
    W_UV = np.asarray(W_UV, np.float32)
    Wq = np.asarray(Wq, np.float32)
    Wo = np.asarray(Wo, np.float32)

    xt = np.ascontiguousarray(x.reshape(S, D).T)          # (D, S)
    wdkvT = np.ascontiguousarray(W_DKV.T)                  # (KV, D)

    cos = freqs_cis[:, :, 0].T                             # (64, S)
    sin = freqs_cis[:, :, 1].T
    cmap = np.repeat(cos, 2, axis=0).astype(np.float32)    # (128, S)
    smap = np.repeat(sin, 2, axis=0).astype(np.float32)
    smap[0::2, :] *= -1.0                                  # rot = [q1->2i(-sin), q0->2i+1(+sin)]

    masks = np.zeros((P, 4, 512), np.float32)
    pp = np.arange(P)[:, None]
    ff = np.arange(512)[None, :]
    for j in range(4):
        masks[:, j, :] = np.where(j * P + pp > ff, BIG_NEG, 0.0)
    masks = masks.reshape(P, 4 * 512)

    consts = np.zeros((P, 385), np.float32)
    consts[:, 0:128] = np.eye(P)
    rsw = np.zeros((P, P), np.float32)
    idx = np.arange(0, P, 2)
    rsw[idx, idx + 1] = 1.0
    rsw[idx + 1, idx] = 1.0
    consts[:, 128:256] = rsw
    consts[:, 256] = 1.0
    consts[0, 257:385] = 1.0

    in_maps = []
    for c in range(NCORES):
        wq_c = np.ascontiguousarray(Wq[:, c * 512:(c + 1) * 512])
        wukv_c = np.ascontiguousarray(
            np.concatenate([W_UK[:, c * HD:(c + 1) * HD],
                            W_UV[:, c * HD:(c + 1) * HD]], axis=1))
        sl = W_DKV[:, c * 64:(c + 1) * 64]                 # (D, 64)
        wdkvsl_c = np.ascontiguousarray(
            sl.reshape(DT, P, 64).transpose(1, 0, 2).reshape(P, DT * 64))
        wo_c = np.ascontiguousarray(Wo[c * 512:(c + 1) * 512, :])
        in_maps.append({
            "xt": xt, "wq": wq_c, "wdkvT": wdkvT, "wukv": wukv_c,
            "wdkvsl": wdkvsl_c, "wo": wo_c, "cmap": cmap, "smap": smap,
            "masks": masks, "consts": consts,
        })
    return in_maps


def _run(inputs, trace=False, tmpdir=None):
    if "nc" not in _CACHE:
        _CACHE["nc"] = _build_nc()
    nc = _CACHE["nc"]
    in_maps = _prep_in_maps(**inputs)
    res = bass_utils.run_bass_kernel_spmd(
        nc, in_maps, core_ids=list(range(NCORES)), trace=trace, tmpdir=tmpdir)
    out = np.zeros((S, D), np.float64)
    for c in range(NCORES):
        out += res.results[c]["outp"].astype(np.float64)
    out = out.astype(np.float32).reshape(1, S, D)
    ckvT = np.concatenate([res.results[c]["ckvT"] for c in range(NCORES)],
                          axis=0)                          # (512, S)
    c_kv = np.ascontiguousarray(ckvT.T).reshape(1, S, KV)
    return (out, c_kv), res


def kernel(**inputs):
    (out, c_kv), _ = _run(inputs, trace=False)
    return out, c_kv


# revision 13
# speedup vs baseline: 1.2784x; 1.2784x over previous
"""MLA (multi-head latent attention) Trainium2 kernel, 8-core tensor-parallel
over heads. Self-contained: hardcodes shapes for
B=1, S=2048, D=4096, NH=32, NKV=8, HD=128, KV_RANK=512.

Sharding: core c owns query heads [4c, 4c+4) and KV head c (GQA N_REP=4 means
those 4 query heads share exactly one KV head). W_DKV@W_UK / W_DKV@W_UV are
fused on-device so K^T/V^T come straight from x^T; each core also emits a
64-column slice of c_KV. The Wo partial products are summed on the host.
"""
import math
import sys

sys.path.insert(0, "/opt/trn_rl_repo")

import numpy as np

import concourse.bacc as bacc
import concourse.mybir as mybir
import concourse.tile as tile
from concourse import bass_utils

P = 128
S = 2048
D = 4096
KV = 512
NH = 32
NKV = 8
HD = 128
NHC = 4              # query heads per core
NCORES = 8
DT = D // P          # 32 d-tiles
CT = KV // P         # 4 rank-tiles
SC = S // 512        # 4 s-chunks
KT = S // P          # 16 k-tiles
ST = S // P          # 16 s-tiles
DC = D // 512        # 8 output column chunks
SCALE = 1.0 / math.sqrt(HD)
BIG_NEG = -60000.0

F32R = mybir.dt.float32r
F32 = mybir.dt.float32
AF = mybir.ActivationFunctionType

_CACHE = {}


def _build_nc():
    nc = bacc.Bacc("TRN2", target_bir_lowering=False, debug=False,
                   num_devices=NCORES)

    d_xt = nc.dram_tensor("xt", (D, S), F32R, kind="ExternalInput").ap()
    d_wq = nc.dram_tensor("wq", (D, 512), F32R, kind="ExternalInput").ap()
    d_wdkvT = nc.dram_tensor("wdkvT", (KV, D), F32R, kind="ExternalInput").ap()
    d_wukv = nc.dram_tensor("wukv", (KV, 256), F32R, kind="ExternalInput").ap()
    # pre-tiled image: [p, dt*64 + c] = W_DKV[dt*128 + p, core*64 + c]
    d_wdkvsl = nc.dram_tensor("wdkvsl", (P, DT * 64), F32R,
                              kind="ExternalInput").ap()
    d_wo = nc.dram_tensor("wo", (512, D), F32R, kind="ExternalInput").ap()
    d_cmap = nc.dram_tensor("cmap", (P, S), F32R, kind="ExternalInput").ap()
    d_smap = nc.dram_tensor("smap", (P, S), F32R, kind="ExternalInput").ap()
    d_masks = nc.dram_tensor("masks", (P, 4 * 512), F32R,
                             kind="ExternalInput").ap()
    # consts image: cols [0,128) identity, [128,256) rope-swap perm,
    # col 256 ones-column; row 0 cols [257,385) ones-row
    d_consts = nc.dram_tensor("consts", (P, 385), F32R,
                              kind="ExternalInput").ap()

    d_out = nc.dram_tensor("outp", (S, D), F32, kind="ExternalOutput").ap()
    d_ckvT = nc.dram_tensor("ckvT", (64, S), F32, kind="ExternalOutput").ap()

    with tile.TileContext(nc) as tc:
        with tc.tile_pool(name="g_const", bufs=1) as gconst, \
             tc.tile_pool(name="persist", bufs=1) as persist:
            consts = gconst.tile([P, 385], F32R)
            nc.sync.dma_start(out=consts, in_=d_consts)
            ident = consts[:, 0:128]
            rswap = consts[:, 128:256]
            onec = consts[:, 256:257]
            oner = consts[0:1, 257:385]

            qT = persist.tile([P, NHC, S], F32R)     # roped Q^T per head
            kT = persist.tile([P, S], F32R)          # roped K^T
            vT = persist.tile([P, S], F32R)          # V^T (pre-transpose)

            # ---------------- Phase 0 + A weights ----------------
            pAw_ctx = tc.tile_pool(name="pA_w", bufs=1)
            with pAw_ctx as pAw:
                wkvf_sb = pAw.tile([P, DT, 256], F32R)
                wdkvsl_sb = pAw.tile([P, DT, 64], F32R)
                wq_sb = pAw.tile([P, DT, 512], F32R)

                # Phase 0: WKV_fused[d, :] = sum_c W_DKV[d, c] * [W_UK|W_UV][c, :]
                with tc.tile_pool(name="ph0", bufs=3) as ph0, \
                     tc.tile_pool(name="wukvp", bufs=1) as wukvp, \
                     tc.tile_pool(name="ps0", bufs=2, space="PSUM") as ps0:
                    wukv_sb = wukvp.tile([P, CT, 256], F32R)
                    nc.sync.dma_start(
                        out=wukv_sb,
                        in_=d_wukv.rearrange("(a p) m -> p a m", p=P))
                    wdkvT_v = d_wdkvT.rearrange("(a p) d -> p a d", p=P)
                    for dtb in range(8):
                        wt = ph0.tile([P, CT, 512], F32R, tag="wdkvT")
                        nc.sync.dma_start(
                            out=wt, in_=wdkvT_v[:, :, dtb * 512:(dtb + 1) * 512])
                        for j in range(4):
                            dt = dtb * 4 + j
                            ps = ps0.tile([P, 256], F32, tag="ps0")
                            for ct in range(CT):
                                nc.tensor.matmul(
                                    ps,
                                    lhsT=wt[:, ct, j * P:(j + 1) * P],
                                    rhs=wukv_sb[:, ct, :],
                                    start=(ct == 0), stop=(ct == CT - 1))
                            nc.scalar.copy(wkvf_sb[:, dt, :], ps)

                nc.sync.dma_start(
                    out=wdkvsl_sb,
                    in_=d_wdkvsl.rearrange("p (a c) -> p a c", a=DT))
                for dt in range(DT):
                    nc.sync.dma_start(out=wq_sb[:, dt, :],
                                      in_=d_wq[dt * P:(dt + 1) * P, :])

                # Phase A: stream x^T, project Q/K/V + c_KV slice, apply rope
                with tc.tile_pool(name="xtp", bufs=6) as xtp, \
                     tc.tile_pool(name="mapp", bufs=2) as mapp, \
                     tc.tile_pool(name="ropep", bufs=1) as ropep, \
                     tc.tile_pool(name="stg", bufs=2) as stg, \
                     tc.tile_pool(name="psA", bufs=1, space="PSUM") as psA, \
                     tc.tile_pool(name="psR", bufs=1, space="PSUM") as psR:
                    for sc in range(SC):
                        s0 = sc * 512
                        qps = [psA.tile([P, 512], F32, tag=f"qps{h}",
                                        name=f"qps{h}_{sc}")
                               for h in range(NHC)]
                        kps = psA.tile([P, 512], F32, tag="kps")
                        vps = psA.tile([P, 512], F32, tag="vps")
                        cps = psA.tile([64, 512], F32, tag="cps")
                        for dt in range(DT):
                            xt_t = xtp.tile([P, 512], F32R, tag="xt")
                            nc.sync.dma_start(
                                out=xt_t,
                                in_=d_xt[dt * P:(dt + 1) * P, s0:s0 + 512])
                            st_f = (dt == 0)
                            sp_f = (dt == DT - 1)
                            for h in range(NHC):
                                nc.tensor.matmul(
                                    qps[h],
                                    lhsT=wq_sb[:, dt, h * P:(h + 1) * P],
                                    rhs=xt_t, start=st_f, stop=sp_f)
                            nc.tensor.matmul(kps, lhsT=wkvf_sb[:, dt, 0:128],
                                             rhs=xt_t, start=st_f, stop=sp_f)
                            nc.tensor.matmul(vps, lhsT=wkvf_sb[:, dt, 128:256],
                                             rhs=xt_t, start=st_f, stop=sp_f)
                            nc.tensor.matmul(cps, lhsT=wdkvsl_sb[:, dt, :],
                                             rhs=xt_t, start=st_f, stop=sp_f)
                        # c_KV slice out (fp32)
                        c_sb = stg.tile([64, 512], F32, tag="ckv")
                        nc.scalar.copy(c_sb, cps)
                        nc.sync.dma_start(out=d_ckvT[:, s0:s0 + 512], in_=c_sb)
                        # V^T evict (no rope)
                        nc.scalar.copy(vT[:, s0:s0 + 512], vps)
                        # rope: dest = cmap*raw + smap*(rswap @ raw)
                        cm = mapp.tile([P, 512], F32R, tag="cm")
                        nc.sync.dma_start(out=cm, in_=d_cmap[:, s0:s0 + 512])
                        sm = mapp.tile([P, 512], F32R, tag="sm")
                        nc.sync.dma_start(out=sm, in_=d_smap[:, s0:s0 + 512])
                        streams = [(qps[h], qT[:, h, s0:s0 + 512])
                                   for h in range(NHC)]
                        streams.append((kps, kT[:, s0:s0 + 512]))
                        for si, (psrc, dest) in enumerate(streams):
                            raw = ropep.tile([P, 512], F32R, tag="raw",
                                             bufs=2)
                            if si % 2 == 0:
                                nc.scalar.copy(raw, psrc)
                            else:
                                nc.vector.tensor_copy(raw, psrc)
                            rps = psR.tile([P, 512], F32, tag="rot")
                            nc.tensor.matmul(rps, lhsT=rswap, rhs=raw,
                                             start=True, stop=True)
                            rot = ropep.tile([P, 512], F32R, tag="rot_sb")
                            if si % 2 == 0:
                                nc.vector.tensor_copy(rot, rps)
                            else:
                                nc.scalar.copy(rot, rps)
                            t1 = ropep.tile([P, 512], F32R, tag="t1")
                            nc.vector.tensor_mul(t1, raw, cm)
                            t2 = ropep.tile([P, 512], F32R, tag="t2")
                            nc.vector.tensor_mul(t2, rot, sm)
                            nc.vector.tensor_add(dest, t1, t2)

            # ---------------- Phase B: attention + fused Wo ----------------
            pBp_cm = tc.tile_pool(name="pB_persist", bufs=1)
            pBp = pBp_cm.__enter__()
            attnT = pBp.tile([P, NHC, S], F32R)
            masks_sb = pBp.tile([P, 4, 512], F32R)
            nc.sync.dma_start(out=masks_sb,
                              in_=d_masks.rearrange("p (a m) -> p a m", a=4))
            wo_sb = pBp.tile([P, 4, D], F32R)
            for ht in range(4):
                nc.sync.dma_start(out=wo_sb[:, ht, :],
                                  in_=d_wo[ht * P:(ht + 1) * P, :])
            v_sb = pBp.tile([P, KT, P], F32R)

            with tc.tile_pool(name="ptp", bufs=4) as ptp, \
                 tc.tile_pool(name="obp", bufs=2) as obp, \
                 tc.tile_pool(name="lrp", bufs=2) as lrp, \
                 tc.tile_pool(name="ocp", bufs=4) as ocp, \
                 tc.tile_pool(name="psB", bufs=1, space="PSUM") as psB:
                # prologue: transpose V^T tiles -> V natural
                for kt in range(KT):
                    tp = psB.tile([P, P], F32R, tag="c", bufs=2,
                                  name=f"tp{kt}")
                    nc.tensor.transpose(tp, vT[:, kt * P:(kt + 1) * P], ident)
                    nc.scalar.copy(v_sb[:, kt, :], tp)

                for qc in range(SC):
                    q0 = qc * 512
                    kts = 4 * (qc + 1)
                    for h in range(NHC):
                        oacc = psB.tile([P, 512], F32, tag="o", bufs=2,
                                        name=f"oacc{h}_{qc}")
                        lacc = psB.tile([1, 512], F32, tag="l", bufs=1,
                                        name=f"lacc{h}_{qc}")
                        for kt in range(kts):
                            sps = psB.tile([P, 512], F32, tag="s", bufs=2,
                                           name=f"sps{h}_{qc}_{kt}")
                            diag = (kt >= 4 * qc)
                            nc.tensor.matmul(
                                sps, lhsT=kT[:, kt * P:(kt + 1) * P],
                                rhs=qT[:, h, q0:q0 + 512],
                                start=True, stop=(not diag))
                            if diag:
                                nc.tensor.matmul(
                                    sps, lhsT=ident,
                                    rhs=masks_sb[:, kt - 4 * qc, :],
                                    start=False, stop=True)
                            pt = ptp.tile([P, 512], F32R, tag="pt")
                            nc.scalar.activation(pt, sps, AF.Exp, scale=SCALE)
                            nc.tensor.matmul(oacc, lhsT=v_sb[:, kt, :], rhs=pt,
                                             start=(kt == 0),
                                             stop=(kt == kts - 1))
                            nc.tensor.matmul(lacc, lhsT=onec, rhs=pt,
                                             start=(kt == 0),
                                             stop=(kt == kts - 1))
                        # r = 1/l on DVE; broadcast via K=1 matmul
                        l_sb = lrp.tile([1, 512], F32, tag="l_sb")
                        nc.vector.tensor_copy(l_sb, lacc)
                        rf = lrp.tile([1, 512], F32, tag="rf")
                        nc.vector.reciprocal_approx_fast(rf, l_sb)
                        rr = lrp.tile([1, 512], F32R, tag="rr")
                        nc.vector.tensor_copy(rr, rf)
                        rbc = psB.tile([P, 512], F32, tag="rbc", bufs=1,
                                       name=f"rbc{h}_{qc}")
                        nc.tensor.matmul(rbc, lhsT=oner, rhs=rr,
                                         start=True, stop=True)
                        o_sb = obp.tile([P, 512], F32, tag="osb")
                        nc.scalar.copy(o_sb, oacc)
                        nc.vector.tensor_mul(attnT[:, h, q0:q0 + 512],
                                             o_sb, rbc)
                    # fused phase C for the s-tiles this qc completed
                    for st in range(4 * qc, 4 * qc + 4):
                        for dc in range(DC):
                            ps = psB.tile([P, 512], F32, tag="c", bufs=2,
                                          name=f"cps{st}_{dc}")
                            for ht in range(4):
                                nc.tensor.matmul(
                                    ps,
                                    lhsT=attnT[:, ht, st * P:(st + 1) * P],
                                    rhs=wo_sb[:, ht, dc * 512:(dc + 1) * 512],
                                    start=(ht == 0), stop=(ht == 3))
                            o2 = ocp.tile([P, 512], F32, tag="oc")
                            nc.any.tensor_copy(o2, ps)
                            nc.sync.dma_start(
                                out=d_out[st * P:(st + 1) * P,
                                          dc * 512:(dc + 1) * 512],
                                in_=o2)
            pBp_cm.__exit__(None, None, None)

    nc.compile()
    return nc


def _prep_in_maps(x, freqs_cis, W_DKV, W_UK, W_UV, Wq, Wo):
    x = np.asarray(x, np.float32)
    freqs_cis = np.asarray(freqs_cis, np.float32)
    W_DKV = np.asarray(W_DKV, np.float32)
    W_UK = np.asarray(W_UK, np.float32)
    W_UV = np.asarray(W_UV, np.float32)
    Wq = np.asarray(Wq, np.float32)
    Wo = np.asarray(Wo, np.float32)

    xt = np.ascontiguousarray(x.reshape(S, D).T)          # (D, S)
    wdkvT = np.ascontiguousarray(W_DKV.T)                  # (KV, D)

    cos = freqs_cis[:, :, 0].T                             # (64, S)
    sin = freqs_cis[:, :, 1].T
    cmap = np.repeat(cos, 2, axis=0).astype(np.float32)    # (128, S)
    smap = np.repeat(sin, 2, axis=0).astype(np.float32)
    smap[0::2, :] *= -1.0                                  # rot = [q1->2i(-sin), q0->2i+1(+sin)]

    masks = np.zeros((P, 4, 512), np.float32)
    pp = np.arange(P)[:, None]
    ff = np.arange(512)[None, :]
    for j in range(4):
        masks[:, j, :] = np.where(j * P + pp > ff, BIG_NEG, 0.0)
    masks = masks.reshape(P, 4 * 512)

    consts = np.zeros((P, 385), np.float32)
    consts[:, 0:128] = np.eye(P)
    rsw = np.zeros((P, P), np.float32)
    idx = np.arange(0, P, 2)
    rsw[idx, idx + 1] = 1.0
    rsw[idx + 1, idx] = 1.0
    consts[:, 128:256] = rsw
    consts[:, 256] = 1.0
    consts[0, 257:385] = 1.0

    in_maps = []
    for c in range(NCORES):
        wq_c = np.ascontiguousarray(Wq[:, c * 512:(c + 1) * 512])
        wukv_c = np.ascontiguousarray(
            np.concatenate([W_UK[:, c * HD:(c + 1) * HD],
                            W_UV[:, c * HD:(c + 1) * HD]], axis=1))
        sl = W_DKV[:, c * 64:(c + 1) * 64]                 # (D, 64)
        wdkvsl_c = np.ascontiguousarray(
            sl.reshape(DT, P, 64).transpose(1, 0, 2).reshape(P, DT * 64))
        wo_c = np.ascontiguousarray(Wo[c * 512:(c + 1) * 512, :])
        in_maps.append({
            "xt": xt, "wq": wq_c, "wdkvT": wdkvT, "wukv": wukv_c,
            "wdkvsl": wdkvsl_c, "wo": wo_c, "cmap": cmap, "smap": smap,
            "masks": masks, "consts": consts,
        })
    return in_maps


def _run(inputs, trace=False, tmpdir=None):
    if "nc" not in _CACHE:
        _CACHE["nc"] = _build_nc()
    nc = _CACHE["nc"]
    in_maps = _prep_in_maps(**inputs)
    res = bass_utils.run_bass_kernel_spmd(
        nc, in_maps, core_ids=list(range(NCORES)), trace=trace, tmpdir=tmpdir)
    out = np.zeros((S, D), np.float64)
    for c in range(NCORES):
        out += res.results[c]["outp"].astype(np.float64)
    out = out.astype(np.float32).reshape(1, S, D)
    ckvT = np.concatenate([res.results[c]["ckvT"] for c in range(NCORES)],
                          axis=0)                          # (512, S)
    c_kv = np.ascontiguousarray(ckvT.T).reshape(1, S, KV)
    return (out, c_kv), res


def kernel(**inputs):
    (out, c_kv), _ = _run(inputs, trace=False)
    return out, c_kv
